# revision 33
# baseline (speedup 1.0000x reference)
"""Gaussian NLL loss kernel for Trainium2 (8 NeuronCores, data-parallel).

out[n] = 0.5 * (x_n - mu)^T pinv(sigma+eps) (x_n - mu) + log_den,  shape [N, 1]

Strategy (v2):
  Host: tiny D x D prep (pinv -> symmetrize -> Cholesky L, slogdet); center
  X by mu and cast to fp16 (halves HBM traffic; rel err ~1e-3 << 2e-2
  budget); one-time transpose so each core's DMA loads are 8KB-contiguous
  per partition (features on partitions).
  Device (per core, N/8 samples, software-pipelined):
    z    = (L/sqrt 2)^T x'         fp16 matmul, stationary = L'
    zsq  = z^2                     ScalarE Square (no bias needed: x is
                                   pre-centered on host), fp16 out
    q    = colsum(zsq)             matmul with one-hot selector columns,
                                   accumulating 8 sub-chunks into one
                                   [8, 512] PSUM tile per chunk
    out  = q + log_den             VectorE, then DMA out
  The selector matmuls for group g are issued DEFER groups late so the PE
  always has independent z-matmuls to run while the ACT square for g is
  still in flight (keeps PE dense, HAM stays warm).
  Pure data-parallel: no collectives.
"""

import math
import sys

import numpy as np

sys.path.insert(0, "/opt/trn_rl_repo")

import concourse.bass as bass
import concourse.bacc as bacc
import concourse.mybir as mybir
import concourse.tile as tile
from concourse.bass_utils import run_bass_kernel_spmd

N, D = 1048576, 128
NCORES = 8
NSH = N // NCORES   # 131072 samples per core
CHUNK = 4096        # samples per DMA tile (1MB fp16 transfers, 8KB/partition)
GROUP = 1024        # samples per ACT square op (2 PSUM banks)
SUB = 512           # samples per matmul (PSUM bank limit)
JPC = CHUNK // SUB  # 8 sub-chunks per chunk
GPC = CHUNK // GROUP  # 4 groups per chunk
NG = NSH // GROUP   # 128 groups per core
NPAIR = NG // 2     # pair-iterations (2 groups each)
DP = 2              # pairs the selector matmuls lag behind the z matmuls
DVE_EVERY = 4       # every DVE_EVERY-th group squares on VectorE (2-op detour)

_f32 = mybir.dt.float32
_f16 = mybir.dt.float16

LAST_RESULTS = None  # BassKernelResults of the most recent run (for test.py)


def _build_bass(log_den: float) -> bass.Bass:
    nc = bacc.Bacc()
    xt = nc.declare_dram_parameter("xt", [D, NSH], _f16, isOutput=False)
    wsel = nc.declare_dram_parameter("wsel", [D, D + JPC * JPC], _f16,
                                     isOutput=False)
    out = nc.declare_dram_parameter("out", [1, NSH], _f32, isOutput=True)

    with tile.TileContext(nc) as tc:
        with (
            tc.tile_pool(name="const", bufs=1) as cpool,
            tc.tile_pool(name="xin", bufs=6) as xpool,
            tc.tile_pool(name="zsq", bufs=2 * DP + 3) as zpool,
            tc.tile_pool(name="zc", bufs=2) as zcpool,
            tc.tile_pool(name="outs", bufs=3) as opool,
            tc.tile_pool(name="pz", bufs=3, space=bass.MemorySpace.PSUM) as pzpool,
            tc.tile_pool(name="pq", bufs=2, space=bass.MemorySpace.PSUM) as pqpool,
        ):
            ws_t = cpool.tile([D, D + JPC * JPC], _f16)
            nc.sync.dma_start(out=ws_t[:], in_=wsel[:])
            lw_t = ws_t[:, 0:D]
            sel_t = ws_t[:, D:D + JPC * JPC]
            # Pre-sync the PE on the packed-constants DMA with one dummy
            # matmul so real matmuls never stall on the const load; also
            # starts the HAM warm-up clock.
            warm = pqpool.tile([JPC, JPC], _f32, tag="pq", name="warm")
            nc.tensor.matmul(warm[:], sel_t[:, 0:JPC], sel_t[:, 0:JPC])
            # Pre-warm the ACT square table (~2.7us load) during the initial
            # DMA fill instead of on the first real square. Operands are
            # scratch; the result is never read.
            aw2 = cpool.tile([D, 1], _f16, name="aw2")
            nc.scalar.activation(
                aw2[:], ws_t[:, 0:1], mybir.ActivationFunctionType.Square
            )

            x_tiles = {}   # chunk -> sbuf tile
            pq_tiles = {}  # chunk -> psum tile
            zq_tiles = {}  # group -> sbuf tile

            def z_and_square(g):
                c = g // GPC
                if g % GPC == 0:
                    x_t = xpool.tile([D, CHUNK], _f16, tag="x", name="x_t")
                    nc.sync.dma_start(
                        out=x_t[:], in_=xt[:, c * CHUNK:(c + 1) * CHUNK]
                    )
                    x_tiles[c] = x_t
                    pq_tiles[c] = pqpool.tile(
                        [JPC, SUB], _f32, tag="pq", name="pq"
                    )
                pz = pzpool.tile([D, GROUP], _f32, tag="pz", name="pz")
                base = (g % GPC) * GROUP
                for s in range(GROUP // SUB):
                    lo = base + s * SUB
                    nc.tensor.matmul(
                        pz[:, s * SUB:(s + 1) * SUB],
                        lw_t,
                        x_tiles[c][:, lo:lo + SUB],
                    )
                zq = zpool.tile([D, GROUP], _f16, tag="zq", name="zq")
                # DVE squares only at positions issued BEFORE the same
                # pair's +log_den add (odd pairs): g%4==3 (second group of
                # every odd pair) and g%8==6 (first group of every other odd
                # pair). g==1 accelerates the pipeline-fill ramp.
                if g % 4 == 3 or g % 8 == 6 or g == 1:
                    # Square on the DVE (ScalarE is the critical engine).
                    # DVE may read only ONE operand from PSUM, so detour:
                    # copy pz -> fp16 SBUF (1x), then fp16 TT square (2x).
                    zc = zcpool.tile([D, GROUP], _f16, tag="zc", name="zc")
                    nc.vector.tensor_copy(zc[:], pz[:])
                    nc.vector.tensor_mul(zq[:], zc[:], zc[:])
                else:
                    nc.scalar.activation(
                        zq[:], pz[:], mybir.ActivationFunctionType.Square
                    )
                zq_tiles[g] = zq  # consumed DP pair-iterations later

            def sel_and_out(gg):
                cc = gg // GPC
                zqq = zq_tiles.pop(gg)
                for s in range(GROUP // SUB):
                    j = (gg % GPC) * (GROUP // SUB) + s
                    nc.tensor.matmul(
                        pq_tiles[cc],
                        sel_t[:, j * JPC:(j + 1) * JPC],
                        zqq[:, s * SUB:(s + 1) * SUB],
                        start=(j == 0),
                        stop=(j == JPC - 1),
                        skip_group_check=True,
                    )
                if gg % GPC == GPC - 1:
                    del x_tiles[cc]
                    o_t = opool.tile([JPC, SUB], _f32, tag="o", name="o_t")
                    # +log_den on DVE. The DVE-squared group sits at g%4==3,
                    # i.e. the SAME pair as this add and issued before it —
                    # so the scheduler cannot interleave the (PE-waiting) add
                    # between that group's copy+mul chain.
                    nc.vector.tensor_scalar_add(
                        o_t[:], pq_tiles.pop(cc)[:], float(log_den)
                    )
                    nc.sync.dma_start(
                        out=out[:, cc * CHUNK:(cc + 1) * CHUNK].rearrange(
                            "a (j n) -> (a j) n", j=JPC
                        ),
                        in_=o_t[:],
                    )

            # Software pipeline in PAIRS of groups: the PE runs 4 z-matmuls
            # back-to-back, then 4 selector matmuls for the pair DP
            # iterations back — halves the stationary-weight swaps and gives
            # the squares 2 pair-periods of slack before their consumers.
            for p in range(NPAIR):
                z_and_square(2 * p)
                z_and_square(2 * p + 1)
                if p >= DP:
                    sel_and_out(2 * (p - DP))
                    sel_and_out(2 * (p - DP) + 1)
            for gg in range(2 * (NPAIR - DP), NG):  # epilogue: flush sels
                sel_and_out(gg)
    nc.compile()
    return nc


def _install_trace_shim():
    """The image lacks ``antenv.axon_hooks``; recreate it and register the
    ctypes NTFF hook that trn_boot would have installed."""
    import types
    import antenv

    if "antenv.axon_hooks" not in sys.modules:
        mod = types.ModuleType("antenv.axon_hooks")
        holder = [None]
        mod.set_axon_ntff_profile_hook = lambda h: holder.__setitem__(0, h)
        mod.get_axon_ntff_profile_hook = lambda: holder[0]
        sys.modules["antenv.axon_hooks"] = mod
        antenv.axon_hooks = mod
    from antenv.axon_hooks import (
        get_axon_ntff_profile_hook,
        set_axon_ntff_profile_hook,
    )

    if get_axon_ntff_profile_hook() is None:
        from trn_agent_boot.trn_boot import _ntff_profile_via_ctypes

        set_axon_ntff_profile_hook(
            _ntff_profile_via_ctypes("/opt/axon/libaxon_pjrt.so")
        )


def kernel(X: np.ndarray, mu: np.ndarray, sigma: np.ndarray, eps: np.ndarray,
           _trace: bool = False) -> np.ndarray:
    global LAST_RESULTS

    # ---- host prep: tiny D x D linear algebra in float64 ----
    sig = (sigma.astype(np.float64) + eps.astype(np.float64))
    S = np.linalg.pinv(sig)
    _, logdet = np.linalg.slogdet(sig)
    log_den = 0.5 * (D * math.log(2.0 * math.pi) + logdet)
    Ssym = 0.5 * (S + S.T)
    L = np.linalg.cholesky(Ssym)          # S = L @ L.T
    Lp = (L / math.sqrt(2.0)).astype(np.float16)  # [d, e] stationary

    sel = np.zeros((D, JPC * JPC), dtype=np.float16)
    for j in range(JPC):
        sel[:, j * JPC + j] = 1.0
    wsel = np.ascontiguousarray(np.concatenate([Lp, sel], axis=1))

    # ---- center by mu on host (removes the device-side bias), cast fp16 ----
    Xc = (X - mu[None, :]).astype(np.float16)
    XT = np.ascontiguousarray(Xc.T)  # [D, N]
    in_maps = []
    for c_id in range(NCORES):
        in_maps.append({
            "xt": np.ascontiguousarray(XT[:, c_id * NSH:(c_id + 1) * NSH]),
            "wsel": wsel,
        })

    nc = _build_bass(log_den)
    if _trace:
        _install_trace_shim()
        import tempfile
        import concourse.bass_utils as _bu
        _bu.upload_artifacts = lambda d: "local://" + d  # no S3 in this container
        tmpdir = tempfile.mkdtemp(prefix="bass_trace_")
        print("trace dir:", tmpdir)
        res = run_bass_kernel_spmd(
            nc, in_maps, list(range(NCORES)), trace=True, tmpdir=tmpdir
        )
    else:
        res = run_bass_kernel_spmd(nc, in_maps, list(range(NCORES)))
    LAST_RESULTS = res

    out = np.empty((N, 1), dtype=np.float32)
    for c_id in range(NCORES):
        out[c_id * NSH:(c_id + 1) * NSH, 0] = res.results[c_id]["out"].reshape(-1)
    return out


# revision 35
# speedup vs baseline: 1.0291x; 1.0291x over previous
"""Gaussian NLL loss kernel for Trainium2 (8 NeuronCores, data-parallel).

out[n] = 0.5 * (x_n - mu)^T pinv(sigma+eps) (x_n - mu) + log_den,  shape [N, 1]

Strategy (v2):
  Host: tiny D x D prep (pinv -> symmetrize -> Cholesky L, slogdet); center
  X by mu and cast to fp16 (halves HBM traffic; rel err ~1e-3 << 2e-2
  budget); one-time transpose so each core's DMA loads are 8KB-contiguous
  per partition (features on partitions).
  Device (per core, N/8 samples, software-pipelined):
    z    = (L/sqrt 2)^T x'         fp16 matmul, stationary = L'
    zsq  = z^2                     ScalarE Square (no bias needed: x is
                                   pre-centered on host), fp16 out
    q    = colsum(zsq)             matmul with one-hot selector columns,
                                   accumulating 8 sub-chunks into one
                                   [8, 512] PSUM tile per chunk
    out  = q + log_den             VectorE, then DMA out
  The selector matmuls for group g are issued DEFER groups late so the PE
  always has independent z-matmuls to run while the ACT square for g is
  still in flight (keeps PE dense, HAM stays warm).
  Pure data-parallel: no collectives.
"""

import math
import sys

import numpy as np

sys.path.insert(0, "/opt/trn_rl_repo")

import concourse.bass as bass
import concourse.bacc as bacc
import concourse.mybir as mybir
import concourse.tile as tile
from concourse.bass_utils import run_bass_kernel_spmd

N, D = 1048576, 128
NCORES = 8
NSH = N // NCORES   # 131072 samples per core
CHUNK = 4096        # samples per DMA tile (1MB fp16 transfers, 8KB/partition)
GROUP = 1024        # samples per ACT square op (2 PSUM banks)
SUB = 512           # samples per matmul (PSUM bank limit)
JPC = CHUNK // SUB  # 8 sub-chunks per chunk
GPC = CHUNK // GROUP  # 4 groups per chunk
NG = NSH // GROUP   # 128 groups per core
NPAIR = NG // 2     # pair-iterations (2 groups each)
DP = 2              # pairs the selector matmuls lag behind the z matmuls
DVE_EVERY = 4       # every DVE_EVERY-th group squares on VectorE (2-op detour)

_f32 = mybir.dt.float32
_f16 = mybir.dt.float16

LAST_RESULTS = None  # BassKernelResults of the most recent run (for test.py)


def _build_bass(log_den: float) -> bass.Bass:
    nc = bacc.Bacc()
    xt = nc.declare_dram_parameter("xt", [D, NSH], _f16, isOutput=False)
    wsel = nc.declare_dram_parameter("wsel", [D, D + JPC * JPC], _f16,
                                     isOutput=False)
    out = nc.declare_dram_parameter("out", [1, NSH], _f32, isOutput=True)

    with tile.TileContext(nc) as tc:
        with (
            tc.tile_pool(name="const", bufs=1) as cpool,
            tc.tile_pool(name="xin", bufs=4) as xpool,
            tc.tile_pool(name="zsq", bufs=2 * DP + 3) as zpool,
            tc.tile_pool(name="zc", bufs=2) as zcpool,
            tc.tile_pool(name="outs", bufs=3) as opool,
            tc.tile_pool(name="pz", bufs=3, space=bass.MemorySpace.PSUM) as pzpool,
            tc.tile_pool(name="pq", bufs=2, space=bass.MemorySpace.PSUM) as pqpool,
        ):
            ws_t = cpool.tile([D, D + JPC * JPC], _f16)
            nc.sync.dma_start(out=ws_t[:], in_=wsel[:])
            lw_t = ws_t[:, 0:D]
            sel_t = ws_t[:, D:D + JPC * JPC]
            # Pre-sync the PE on the packed-constants DMA with one dummy
            # matmul so real matmuls never stall on the const load; also
            # starts the HAM warm-up clock.
            warm = pqpool.tile([JPC, JPC], _f32, tag="pq", name="warm")
            nc.tensor.matmul(warm[:], sel_t[:, 0:JPC], sel_t[:, 0:JPC])
            # Pre-warm the ACT square table (~2.7us load) during the initial
            # DMA fill instead of on the first real square. Operands are
            # scratch; the result is never read.
            aw2 = cpool.tile([D, 1], _f16, name="aw2")
            nc.scalar.activation(
                aw2[:], ws_t[:, 0:1], mybir.ActivationFunctionType.Square
            )

            x_tiles = {}   # chunk -> sbuf tile
            pq_tiles = {}  # chunk -> psum tile
            zq_tiles = {}  # group -> sbuf tile

            def z_and_square(g):
                c = g // GPC
                if g % GPC == 0:
                    x_t = xpool.tile([D, CHUNK], _f16, tag="x", name="x_t")
                    nc.sync.dma_start(
                        out=x_t[:], in_=xt[:, c * CHUNK:(c + 1) * CHUNK]
                    )
                    x_tiles[c] = x_t
                    pq_tiles[c] = pqpool.tile(
                        [JPC, SUB], _f32, tag="pq", name="pq"
                    )
                pz = pzpool.tile([D, GROUP], _f32, tag="pz", name="pz")
                base = (g % GPC) * GROUP
                for s in range(GROUP // SUB):
                    lo = base + s * SUB
                    nc.tensor.matmul(
                        pz[:, s * SUB:(s + 1) * SUB],
                        lw_t,
                        x_tiles[c][:, lo:lo + SUB],
                    )
                zq = zpool.tile([D, GROUP], _f16, tag="zq", name="zq")
                # DVE squares only at g%4==3: the second group of every odd
                # pair, issued BEFORE that pair's +log_den add, so the
                # scheduler can never interleave the (PE-waiting) add into
                # the copy+mul chain. g==1 accelerates the pipeline fill.
                # (Adding g%8==6 measured faster on some runs but with much
                # higher run-to-run variance — 150.4/158.0 vs stable 152.7.)
                if g % 4 == 3 or g == 1:
                    # Square on the DVE (ScalarE is the critical engine).
                    # DVE may read only ONE operand from PSUM, so detour:
                    # copy pz -> fp16 SBUF (1x), then fp16 TT square (2x).
                    zc = zcpool.tile([D, GROUP], _f16, tag="zc", name="zc")
                    nc.vector.tensor_copy(zc[:], pz[:])
                    nc.vector.tensor_mul(zq[:], zc[:], zc[:])
                else:
                    nc.scalar.activation(
                        zq[:], pz[:], mybir.ActivationFunctionType.Square
                    )
                zq_tiles[g] = zq  # consumed DP pair-iterations later

            def sel_and_out(gg):
                cc = gg // GPC
                zqq = zq_tiles.pop(gg)
                for s in range(GROUP // SUB):
                    j = (gg % GPC) * (GROUP // SUB) + s
                    nc.tensor.matmul(
                        pq_tiles[cc],
                        sel_t[:, j * JPC:(j + 1) * JPC],
                        zqq[:, s * SUB:(s + 1) * SUB],
                        start=(j == 0),
                        stop=(j == JPC - 1),
                        skip_group_check=True,
                    )
                if gg % GPC == GPC - 1:
                    del x_tiles[cc]
                    o_t = opool.tile([JPC, SUB], _f32, tag="o", name="o_t")
                    # +log_den on DVE. The DVE-squared group sits at g%4==3,
                    # i.e. the SAME pair as this add and issued before it —
                    # so the scheduler cannot interleave the (PE-waiting) add
                    # between that group's copy+mul chain.
                    nc.vector.tensor_scalar_add(
                        o_t[:], pq_tiles.pop(cc)[:], float(log_den)
                    )
                    nc.sync.dma_start(
                        out=out[:, cc * CHUNK:(cc + 1) * CHUNK].rearrange(
                            "a (j n) -> (a j) n", j=JPC
                        ),
                        in_=o_t[:],
                    )

            # Software pipeline in PAIRS of groups: the PE runs 4 z-matmuls
            # back-to-back, then 4 selector matmuls for the pair DP
            # iterations back — halves the stationary-weight swaps and gives
            # the squares 2 pair-periods of slack before their consumers.
            for p in range(NPAIR):
                z_and_square(2 * p)
                z_and_square(2 * p + 1)
                if p >= DP:
                    sel_and_out(2 * (p - DP))
                    sel_and_out(2 * (p - DP) + 1)
            for gg in range(2 * (NPAIR - DP), NG):  # epilogue: flush sels
                sel_and_out(gg)
    nc.compile()
    return nc


def _install_trace_shim():
    """The image lacks ``antenv.axon_hooks``; recreate it and register the
    ctypes NTFF hook that trn_boot would have installed."""
    import types
    import antenv

    if "antenv.axon_hooks" not in sys.modules:
        mod = types.ModuleType("antenv.axon_hooks")
        holder = [None]
        mod.set_axon_ntff_profile_hook = lambda h: holder.__setitem__(0, h)
        mod.get_axon_ntff_profile_hook = lambda: holder[0]
        sys.modules["antenv.axon_hooks"] = mod
        antenv.axon_hooks = mod
    from antenv.axon_hooks import (
        get_axon_ntff_profile_hook,
        set_axon_ntff_profile_hook,
    )

    if get_axon_ntff_profile_hook() is None:
        from trn_agent_boot.trn_boot import _ntff_profile_via_ctypes

        set_axon_ntff_profile_hook(
            _ntff_profile_via_ctypes("/opt/axon/libaxon_pjrt.so")
        )


def kernel(X: np.ndarray, mu: np.ndarray, sigma: np.ndarray, eps: np.ndarray,
           _trace: bool = False) -> np.ndarray:
    global LAST_RESULTS

    # ---- host prep: tiny D x D linear algebra in float64 ----
    sig = (sigma.astype(np.float64) + eps.astype(np.float64))
    S = np.linalg.pinv(sig)
    _, logdet = np.linalg.slogdet(sig)
    log_den = 0.5 * (D * math.log(2.0 * math.pi) + logdet)
    Ssym = 0.5 * (S + S.T)
    L = np.linalg.cholesky(Ssym)          # S = L @ L.T
    Lp = (L / math.sqrt(2.0)).astype(np.float16)  # [d, e] stationary

    sel = np.zeros((D, JPC * JPC), dtype=np.float16)
    for j in range(JPC):
        sel[:, j * JPC + j] = 1.0
    wsel = np.ascontiguousarray(np.concatenate([Lp, sel], axis=1))

    # ---- center by mu on host (removes the device-side bias), cast fp16 ----
    Xc = (X - mu[None, :]).astype(np.float16)
    XT = np.ascontiguousarray(Xc.T)  # [D, N]
    in_maps = []
    for c_id in range(NCORES):
        in_maps.append({
            "xt": np.ascontiguousarray(XT[:, c_id * NSH:(c_id + 1) * NSH]),
            "wsel": wsel,
        })

    nc = _build_bass(log_den)
    if _trace:
        _install_trace_shim()
        import tempfile
        import concourse.bass_utils as _bu
        _bu.upload_artifacts = lambda d: "local://" + d  # no S3 in this container
        tmpdir = tempfile.mkdtemp(prefix="bass_trace_")
        print("trace dir:", tmpdir)
        res = run_bass_kernel_spmd(
            nc, in_maps, list(range(NCORES)), trace=True, tmpdir=tmpdir
        )
    else:
        res = run_bass_kernel_spmd(nc, in_maps, list(range(NCORES)))
    LAST_RESULTS = res

    out = np.empty((N, 1), dtype=np.float32)
    for c_id in range(NCORES):
        out[c_id * NSH:(c_id + 1) * NSH, 0] = res.results[c_id]["out"].reshape(-1)
    return out
